# revision 3
# baseline (speedup 1.0000x reference)
"""PiInterferenceLayer Trainium2 kernel.

out[b, p] = |sum_d conj(x[b, d]) * (patterns[p, d] * e^{i*phase_p})|^2

Decomposed into real GEMMs with host-side phase folding:
  sr = patterns_real*cos(phi) - patterns_imag*sin(phi)   # [P, D]
  si = patterns_real*sin(phi) + patterns_imag*cos(phi)
  Zr = xr @ sr.T + xi @ si.T       # [B, P]
  Zi = xr @ si.T - xi @ sr.T
  out = Zr^2 + Zi^2

Sharding: batch (8192) split 8 ways across NeuronCores, patterns replicated.
Device kernel (per core, B_c = 1024):
  - x-side operands (xrT, xiT, -xiT) transposed on host to [D, B_c], fully
    resident in SBUF.
  - pattern operands (srT, siT) transposed on host to [D, P], streamed per
    512-pattern slice, double buffered.
  - TensorE fp32r (FP22) matmuls accumulate Zr and Zi in PSUM; ScalarE
    squares PSUM->SBUF; VectorE adds; DMA out.
"""

from contextlib import ExitStack

import numpy as np

import concourse.bass as bass
import concourse.mybir as mybir
import concourse.tile as tile
from concourse import bacc
from concourse.bass_utils import run_bass_kernel_spmd

B = 8192
D = 1024
P = 4096
NCORES = 8
BC = B // NCORES  # rows of x per core

KT = D // 128  # contraction tiles
NBT = BC // 128  # batch tiles per core
PS = 512  # pattern slice (one PSUM bank of fp32)
NPS = P // PS

_prog_cache = {}


def build_program():
    if "nc" in _prog_cache:
        return _prog_cache["nc"]

    nc = bacc.Bacc(
        "TRN2",
        target_bir_lowering=False,
        debug=False,
        enable_asserts=False,
        num_devices=NCORES,
    )
    f32 = mybir.dt.float32
    f32r = mybir.dt.float32r

    xrT = nc.dram_tensor("xrT", [D, BC], f32, kind="ExternalInput").ap()
    xiT = nc.dram_tensor("xiT", [D, BC], f32, kind="ExternalInput").ap()
    nxiT = nc.dram_tensor("nxiT", [D, BC], f32, kind="ExternalInput").ap()
    srT = nc.dram_tensor("srT", [D, P], f32, kind="ExternalInput").ap()
    siT = nc.dram_tensor("siT", [D, P], f32, kind="ExternalInput").ap()
    out = nc.dram_tensor("out", [BC, P], f32, kind="ExternalOutput").ap()

    with ExitStack() as ctx:
        tc = ctx.enter_context(tile.TileContext(nc))

        xpool = ctx.enter_context(tc.tile_pool(name="xres", bufs=1))
        spool = ctx.enter_context(tc.tile_pool(name="pat", bufs=2))
        pr_pool = ctx.enter_context(tc.tile_pool(name="psr", bufs=4, space="PSUM"))
        pi_pool = ctx.enter_context(tc.tile_pool(name="psi", bufs=4, space="PSUM"))
        sq_pool = ctx.enter_context(tc.tile_pool(name="sq", bufs=4))
        o_pool = ctx.enter_context(tc.tile_pool(name="osb", bufs=4))

        # Resident x-side: [128, KT, BC] each (32KB/partition), 3 arrays.
        xr_sb = xpool.tile([128, KT, BC], f32r, tag="xr", name="xr_sb")
        xi_sb = xpool.tile([128, KT, BC], f32r, tag="xi", name="xi_sb")
        nxi_sb = xpool.tile([128, KT, BC], f32r, tag="nxi", name="nxi_sb")
        for k in range(KT):
            ksl = slice(k * 128, (k + 1) * 128)
            nc.sync.dma_start(xr_sb[:, k, :], xrT[ksl, :].bitcast(f32r))
            nc.sync.dma_start(xi_sb[:, k, :], xiT[ksl, :].bitcast(f32r))
            nc.sync.dma_start(nxi_sb[:, k, :], nxiT[ksl, :].bitcast(f32r))

        for p in range(NPS):
            psl = slice(p * PS, (p + 1) * PS)
            sr_sb = spool.tile([128, KT, PS], f32r, tag="sr", name="sr_sb")
            si_sb = spool.tile([128, KT, PS], f32r, tag="si", name="si_sb")
            for k in range(KT):
                ksl = slice(k * 128, (k + 1) * 128)
                nc.sync.dma_start(sr_sb[:, k, :], srT[ksl, psl].bitcast(f32r))
                nc.sync.dma_start(si_sb[:, k, :], siT[ksl, psl].bitcast(f32r))

            for b in range(NBT):
                bsl = slice(b * 128, (b + 1) * 128)
                ps_r = pr_pool.tile([128, PS], f32, name="ps_r")
                ps_i = pi_pool.tile([128, PS], f32, name="ps_i")
                for k in range(KT):
                    first = k == 0
                    last = k == KT - 1
                    xr_t = xr_sb[:, k, bsl]
                    xi_t = xi_sb[:, k, bsl]
                    nxi_t = nxi_sb[:, k, bsl]
                    sr_t = sr_sb[:, k, :]
                    si_t = si_sb[:, k, :]
                    # Zr += xr.T@sr + xi.T@si ; Zi += xr.T@si + (-xi).T@sr
                    nc.tensor.matmul(ps_r[:], xr_t, sr_t, start=first, stop=False)
                    nc.tensor.matmul(ps_i[:], xr_t, si_t, start=first, stop=False)
                    nc.tensor.matmul(ps_r[:], xi_t, si_t, start=False, stop=last)
                    nc.tensor.matmul(ps_i[:], nxi_t, sr_t, start=False, stop=last)

                sq_r = sq_pool.tile([128, PS], f32, tag="sqr", name="sq_r")
                sq_i = sq_pool.tile([128, PS], f32, tag="sqi", name="sq_i")
                nc.scalar.square(sq_r[:], ps_r[:])
                nc.scalar.square(sq_i[:], ps_i[:])
                o_sb = o_pool.tile([128, PS], f32, name="o_sb")
                nc.vector.tensor_add(o_sb[:], sq_r[:], sq_i[:])
                nc.gpsimd.dma_start(out[bsl, psl], o_sb[:])

    nc.compile()
    _prog_cache["nc"] = nc
    return nc


def _prepare_in_maps(x_real, x_imag, patterns_real, patterns_imag, phases):
    x_real = np.ascontiguousarray(np.asarray(x_real, dtype=np.float32))
    x_imag = np.ascontiguousarray(np.asarray(x_imag, dtype=np.float32))
    patterns_real = np.asarray(patterns_real, dtype=np.float32)
    patterns_imag = np.asarray(patterns_imag, dtype=np.float32)
    ph = np.asarray(phases, dtype=np.float64)

    cos = np.cos(ph)[:, None]
    sin = np.sin(ph)[:, None]
    pr = patterns_real.astype(np.float64)
    pi = patterns_imag.astype(np.float64)
    sr = (pr * cos - pi * sin).astype(np.float32)  # [P, D]
    si = (pr * sin + pi * cos).astype(np.float32)
    srT = np.ascontiguousarray(sr.T)  # [D, P]
    siT = np.ascontiguousarray(si.T)

    in_maps = []
    for c in range(NCORES):
        rows = slice(c * BC, (c + 1) * BC)
        xs = x_real[rows]
        xis = x_imag[rows]
        in_maps.append(
            {
                "xrT": np.ascontiguousarray(xs.T),
                "xiT": np.ascontiguousarray(xis.T),
                "nxiT": np.ascontiguousarray(-xis.T),
                "srT": srT,
                "siT": siT,
            }
        )
    return in_maps


def run(inputs, trace=False, **trace_kwargs):
    """Build + run on 8 cores. Returns (full_output, BassKernelResults)."""
    in_maps = _prepare_in_maps(**inputs)
    nc = build_program()
    res = run_bass_kernel_spmd(
        nc, in_maps, list(range(NCORES)), trace=trace, **trace_kwargs
    )
    full = np.concatenate([r["out"] for r in res.results], axis=0)
    return full, res


def kernel(x_real, x_imag, patterns_real, patterns_imag, phases):
    full, _ = run(
        {
            "x_real": x_real,
            "x_imag": x_imag,
            "patterns_real": patterns_real,
            "patterns_imag": patterns_imag,
            "phases": phases,
        }
    )
    return full


# revision 6
# speedup vs baseline: 1.0131x; 1.0131x over previous
"""PiInterferenceLayer Trainium2 kernel.

out[b, p] = |sum_d conj(x[b, d]) * (patterns[p, d] * e^{i*phase_p})|^2

Decomposed into real GEMMs with host-side phase folding:
  sr = patterns_real*cos(phi) - patterns_imag*sin(phi)   # [P, D]
  si = patterns_real*sin(phi) + patterns_imag*cos(phi)
  Zr = xr @ sr.T + xi @ si.T       # [B, P]
  Zi = xr @ si.T - xi @ sr.T
  out = Zr^2 + Zi^2

Sharding: batch (8192) split 8 ways across NeuronCores, patterns replicated.
Device kernel (per core, B_c = 1024):
  - x-side operands (xrT, xiT, -xiT) transposed on host to [D, B_c], fully
    resident in SBUF.
  - pattern operands (srT, siT) transposed on host to [D, P], streamed per
    512-pattern slice, double buffered.
  - TensorE fp32r (FP22) matmuls accumulate Zr and Zi in PSUM; ScalarE
    squares PSUM->SBUF; VectorE adds; DMA out.
"""

from contextlib import ExitStack

import numpy as np

import concourse.bass as bass
import concourse.mybir as mybir
import concourse.tile as tile
from concourse import bacc
from concourse.bass_utils import run_bass_kernel_spmd

B = 8192
D = 1024
P = 4096
NCORES = 8
BC = B // NCORES  # rows of x per core

KT = D // 128  # contraction tiles
NBT = BC // 128  # batch tiles per core
PS = 512  # pattern slice (one PSUM bank of fp32)
NPS = P // PS

_prog_cache = {}


def build_program():
    if "nc" in _prog_cache:
        return _prog_cache["nc"]

    nc = bacc.Bacc(
        "TRN2",
        target_bir_lowering=False,
        debug=False,
        enable_asserts=False,
        num_devices=NCORES,
    )
    f32 = mybir.dt.float32
    f32r = mybir.dt.float32r

    xrT = nc.dram_tensor("xrT", [D, BC], f32, kind="ExternalInput").ap()
    xiT = nc.dram_tensor("xiT", [D, BC], f32, kind="ExternalInput").ap()
    nxiT = nc.dram_tensor("nxiT", [D, BC], f32, kind="ExternalInput").ap()
    srT = nc.dram_tensor("srT", [D, P], f32, kind="ExternalInput").ap()
    siT = nc.dram_tensor("siT", [D, P], f32, kind="ExternalInput").ap()
    out = nc.dram_tensor("out", [BC, P], f32, kind="ExternalOutput").ap()

    with ExitStack() as ctx:
        tc = ctx.enter_context(tile.TileContext(nc))

        xpool = ctx.enter_context(tc.tile_pool(name="xres", bufs=1))
        spool = ctx.enter_context(tc.tile_pool(name="pat", bufs=2))
        pr_pool = ctx.enter_context(tc.tile_pool(name="psr", bufs=4, space="PSUM"))
        pi_pool = ctx.enter_context(tc.tile_pool(name="psi", bufs=4, space="PSUM"))
        sq_pool = ctx.enter_context(tc.tile_pool(name="sq", bufs=4))
        o_pool = ctx.enter_context(tc.tile_pool(name="osb", bufs=4))

        # Resident x-side: [128, KT, BC] each (32KB/partition), 3 arrays.
        xr_sb = xpool.tile([128, KT, BC], f32r, tag="xr", name="xr_sb")
        xi_sb = xpool.tile([128, KT, BC], f32r, tag="xi", name="xi_sb")
        nxi_sb = xpool.tile([128, KT, BC], f32r, tag="nxi", name="nxi_sb")
        for k in range(KT):
            ksl = slice(k * 128, (k + 1) * 128)
            nc.scalar.dma_start(xr_sb[:, k, :], xrT[ksl, :].bitcast(f32r))
            nc.scalar.dma_start(xi_sb[:, k, :], xiT[ksl, :].bitcast(f32r))
            nc.scalar.dma_start(nxi_sb[:, k, :], nxiT[ksl, :].bitcast(f32r))

        for p in range(NPS):
            psl = slice(p * PS, (p + 1) * PS)
            sr_sb = spool.tile([128, KT, PS], f32r, tag="sr", name="sr_sb")
            si_sb = spool.tile([128, KT, PS], f32r, tag="si", name="si_sb")
            for k in range(KT):
                ksl = slice(k * 128, (k + 1) * 128)
                nc.sync.dma_start(sr_sb[:, k, :], srT[ksl, psl].bitcast(f32r))
                nc.sync.dma_start(si_sb[:, k, :], siT[ksl, psl].bitcast(f32r))

            for b in range(NBT):
                bsl = slice(b * 128, (b + 1) * 128)
                ps_r = pr_pool.tile([128, PS], f32, name="ps_r")
                ps_i = pi_pool.tile([128, PS], f32, name="ps_i")
                for k in range(KT):
                    first = k == 0
                    last = k == KT - 1
                    xr_t = xr_sb[:, k, bsl]
                    xi_t = xi_sb[:, k, bsl]
                    nxi_t = nxi_sb[:, k, bsl]
                    sr_t = sr_sb[:, k, :]
                    si_t = si_sb[:, k, :]
                    # Zr += xr.T@sr + xi.T@si ; Zi += xr.T@si + (-xi).T@sr
                    nc.tensor.matmul(ps_r[:], xr_t, sr_t, start=first, stop=False)
                    nc.tensor.matmul(ps_i[:], xr_t, si_t, start=first, stop=False)
                    nc.tensor.matmul(ps_r[:], xi_t, si_t, start=False, stop=last)
                    nc.tensor.matmul(ps_i[:], nxi_t, sr_t, start=False, stop=last)

                # Epilogue on VectorE only (ScalarE ACTIVATE is ~9x slower):
                # copy PSUM->SBUF, square in SBUF (2x fp32 mode), add.
                c_r = sq_pool.tile([128, PS], f32, tag="cr", name="c_r")
                c_i = sq_pool.tile([128, PS], f32, tag="ci", name="c_i")
                nc.vector.tensor_copy(c_r[:], ps_r[:])
                nc.vector.tensor_copy(c_i[:], ps_i[:])
                sq_r = sq_pool.tile([128, PS], f32, tag="sqr", name="sq_r")
                nc.vector.tensor_mul(sq_r[:], c_r[:], c_r[:])
                sq_i = sq_pool.tile([128, PS], f32, tag="sqi", name="sq_i")
                nc.vector.tensor_mul(sq_i[:], c_i[:], c_i[:])
                o_sb = o_pool.tile([128, PS], f32, name="o_sb")
                nc.vector.tensor_add(o_sb[:], sq_r[:], sq_i[:])
                nc.gpsimd.dma_start(out[bsl, psl], o_sb[:])

    nc.compile()
    _prog_cache["nc"] = nc
    return nc


def _prepare_in_maps(x_real, x_imag, patterns_real, patterns_imag, phases):
    x_real = np.ascontiguousarray(np.asarray(x_real, dtype=np.float32))
    x_imag = np.ascontiguousarray(np.asarray(x_imag, dtype=np.float32))
    patterns_real = np.asarray(patterns_real, dtype=np.float32)
    patterns_imag = np.asarray(patterns_imag, dtype=np.float32)
    ph = np.asarray(phases, dtype=np.float64)

    cos = np.cos(ph)[:, None]
    sin = np.sin(ph)[:, None]
    pr = patterns_real.astype(np.float64)
    pi = patterns_imag.astype(np.float64)
    sr = (pr * cos - pi * sin).astype(np.float32)  # [P, D]
    si = (pr * sin + pi * cos).astype(np.float32)
    srT = np.ascontiguousarray(sr.T)  # [D, P]
    siT = np.ascontiguousarray(si.T)

    in_maps = []
    for c in range(NCORES):
        rows = slice(c * BC, (c + 1) * BC)
        xs = x_real[rows]
        xis = x_imag[rows]
        in_maps.append(
            {
                "xrT": np.ascontiguousarray(xs.T),
                "xiT": np.ascontiguousarray(xis.T),
                "nxiT": np.ascontiguousarray(-xis.T),
                "srT": srT,
                "siT": siT,
            }
        )
    return in_maps


def run(inputs, trace=False, **trace_kwargs):
    """Build + run on 8 cores. Returns (full_output, BassKernelResults)."""
    in_maps = _prepare_in_maps(**inputs)
    nc = build_program()
    res = run_bass_kernel_spmd(
        nc, in_maps, list(range(NCORES)), trace=trace, **trace_kwargs
    )
    full = np.concatenate([r["out"] for r in res.results], axis=0)
    return full, res


def kernel(x_real, x_imag, patterns_real, patterns_imag, phases):
    full, _ = run(
        {
            "x_real": x_real,
            "x_imag": x_imag,
            "patterns_real": patterns_real,
            "patterns_imag": patterns_imag,
            "phases": phases,
        }
    )
    return full


# revision 7
# speedup vs baseline: 1.2552x; 1.2390x over previous
"""PiInterferenceLayer Trainium2 kernel.

out[b, p] = |sum_d conj(x[b, d]) * (patterns[p, d] * e^{i*phase_p})|^2

Host folds the phases into the patterns:
  sr = patterns_real*cos(phi) - patterns_imag*sin(phi)   # [P, D]
  si = patterns_real*sin(phi) + patterns_imag*cos(phi)
  Zr = xr @ sr.T + xi @ si.T       # [B, P]
  Zi = xr @ si.T - xi @ sr.T
  out = Zr^2 + Zi^2

3-multiplication complex GEMM (Karatsuba / Gauss) with a = xr, b = -xi,
c = sr, d = si:
  m1 = xr @ sr.T
  m2p = xi @ si.T                      (= -m2)
  m3 = (xr - xi) @ (sr + si).T
  Zr = m1 + m2p
  Zi = m3 - m1 + m2p

Sharding: batch (8192) split 8 ways across NeuronCores, patterns replicated.
Device computes the TRANSPOSED output outT[p, b] per core (patterns are the
matmul stationary side, x is the moving side); host transposes back.

Per core (B_c = 1024):
  - moving x-side (xrT, xiT, (xr-xi)T as [D, B_c]) fully resident in SBUF
  - stationary pattern tiles ([D, P] arrays, 128-pattern columns) streamed,
    double buffered
  - TensorE fp32r (FP22) matmuls accumulate m1/m2p/m3 in PSUM over the
    contraction; VectorE combines + squares; DMA out.
"""

from contextlib import ExitStack

import numpy as np

import concourse.bass as bass
import concourse.mybir as mybir
import concourse.tile as tile
from concourse import bacc
from concourse.bass_utils import run_bass_kernel_spmd

B = 8192
D = 1024
P = 4096
NCORES = 8
BC = B // NCORES  # rows of x per core

KT = D // 128  # contraction tiles
NPT = P // 128  # pattern tiles (stationary, 128 wide)
BCH = 512  # moving-side batch chunk (one PSUM bank of fp32)
NBCH = BC // BCH

_prog_cache = {}


def build_program():
    if "nc" in _prog_cache:
        return _prog_cache["nc"]

    nc = bacc.Bacc(
        "TRN2",
        target_bir_lowering=False,
        debug=False,
        enable_asserts=False,
        num_devices=NCORES,
    )
    f32 = mybir.dt.float32
    f32r = mybir.dt.float32r

    xrT = nc.dram_tensor("xrT", [D, BC], f32, kind="ExternalInput").ap()
    xiT = nc.dram_tensor("xiT", [D, BC], f32, kind="ExternalInput").ap()
    wT = nc.dram_tensor("wT", [D, BC], f32, kind="ExternalInput").ap()
    srT = nc.dram_tensor("srT", [D, P], f32, kind="ExternalInput").ap()
    siT = nc.dram_tensor("siT", [D, P], f32, kind="ExternalInput").ap()
    ssT = nc.dram_tensor("ssT", [D, P], f32, kind="ExternalInput").ap()
    out = nc.dram_tensor("out", [P, BC], f32, kind="ExternalOutput").ap()

    # [D, P] viewed as [q=partition, k, p] for single-DMA stripe loads
    srT_v = srT.bitcast(f32r).rearrange("(k q) p -> q k p", q=128)
    siT_v = siT.bitcast(f32r).rearrange("(k q) p -> q k p", q=128)
    ssT_v = ssT.bitcast(f32r).rearrange("(k q) p -> q k p", q=128)

    with ExitStack() as ctx:
        tc = ctx.enter_context(tile.TileContext(nc))

        xpool = ctx.enter_context(tc.tile_pool(name="xres", bufs=1))
        spool = ctx.enter_context(tc.tile_pool(name="pat", bufs=3))
        ps1_pool = ctx.enter_context(tc.tile_pool(name="ps1", bufs=2, space="PSUM"))
        ps2_pool = ctx.enter_context(tc.tile_pool(name="ps2", bufs=2, space="PSUM"))
        ps3_pool = ctx.enter_context(tc.tile_pool(name="ps3", bufs=2, space="PSUM"))
        e_pool = ctx.enter_context(tc.tile_pool(name="epi", bufs=2))
        o_pool = ctx.enter_context(tc.tile_pool(name="osb", bufs=3))

        # Resident moving-side x: [128, KT, BC] each (32KB/partition), 3 arrays.
        xr_sb = xpool.tile([128, KT, BC], f32r, tag="xr", name="xr_sb")
        xi_sb = xpool.tile([128, KT, BC], f32r, tag="xi", name="xi_sb")
        w_sb = xpool.tile([128, KT, BC], f32r, tag="w", name="w_sb")
        for k in range(KT):
            ksl = slice(k * 128, (k + 1) * 128)
            nc.scalar.dma_start(xr_sb[:, k, :], xrT[ksl, :].bitcast(f32r))
            nc.scalar.dma_start(xi_sb[:, k, :], xiT[ksl, :].bitcast(f32r))
            nc.scalar.dma_start(w_sb[:, k, :], wT[ksl, :].bitcast(f32r))

        for pt in range(NPT):
            psl = slice(pt * 128, (pt + 1) * 128)
            sr_sb = spool.tile([128, KT, 128], f32r, tag="sr", name="sr_sb")
            si_sb = spool.tile([128, KT, 128], f32r, tag="si", name="si_sb")
            ss_sb = spool.tile([128, KT, 128], f32r, tag="ss", name="ss_sb")
            nc.sync.dma_start(sr_sb[:], srT_v[:, :, psl])
            nc.sync.dma_start(si_sb[:], siT_v[:, :, psl])
            nc.sync.dma_start(ss_sb[:], ssT_v[:, :, psl])

            for bc in range(NBCH):
                bsl = slice(bc * BCH, (bc + 1) * BCH)
                ps1 = ps1_pool.tile([128, BCH], f32, name="ps1")
                ps2 = ps2_pool.tile([128, BCH], f32, name="ps2")
                ps3 = ps3_pool.tile([128, BCH], f32, name="ps3")
                for k in range(KT):
                    first = k == 0
                    last = k == KT - 1
                    nc.tensor.matmul(
                        ps1[:], sr_sb[:, k, :], xr_sb[:, k, bsl], start=first, stop=last
                    )
                    nc.tensor.matmul(
                        ps2[:], si_sb[:, k, :], xi_sb[:, k, bsl], start=first, stop=last
                    )
                    nc.tensor.matmul(
                        ps3[:], ss_sb[:, k, :], w_sb[:, k, bsl], start=first, stop=last
                    )

                # Zr = m1 + m2p ; Zi = m3 - m1 + m2p ; out = Zr^2 + Zi^2
                c1 = e_pool.tile([128, BCH], f32, tag="c1", name="c1")
                nc.vector.tensor_copy(c1[:], ps1[:])
                zr = e_pool.tile([128, BCH], f32, tag="zr", name="zr")
                nc.vector.tensor_add(zr[:], c1[:], ps2[:])
                t3 = e_pool.tile([128, BCH], f32, tag="t3", name="t3")
                nc.vector.tensor_sub(t3[:], ps3[:], c1[:])
                zi = e_pool.tile([128, BCH], f32, tag="zi", name="zi")
                nc.vector.tensor_add(zi[:], t3[:], ps2[:])
                sq_r = e_pool.tile([128, BCH], f32, tag="sqr", name="sq_r")
                nc.vector.tensor_mul(sq_r[:], zr[:], zr[:])
                sq_i = e_pool.tile([128, BCH], f32, tag="sqi", name="sq_i")
                nc.vector.tensor_mul(sq_i[:], zi[:], zi[:])
                o_sb = o_pool.tile([128, BCH], f32, name="o_sb")
                nc.vector.tensor_add(o_sb[:], sq_r[:], sq_i[:])
                nc.gpsimd.dma_start(out[psl, bsl], o_sb[:])

    nc.compile()
    _prog_cache["nc"] = nc
    return nc


def _prepare_in_maps(x_real, x_imag, patterns_real, patterns_imag, phases):
    x_real = np.ascontiguousarray(np.asarray(x_real, dtype=np.float32))
    x_imag = np.ascontiguousarray(np.asarray(x_imag, dtype=np.float32))
    patterns_real = np.asarray(patterns_real, dtype=np.float32)
    patterns_imag = np.asarray(patterns_imag, dtype=np.float32)
    ph = np.asarray(phases, dtype=np.float64)

    cos = np.cos(ph)[:, None]
    sin = np.sin(ph)[:, None]
    pr = patterns_real.astype(np.float64)
    pi = patterns_imag.astype(np.float64)
    sr = (pr * cos - pi * sin).astype(np.float32)  # [P, D]
    si = (pr * sin + pi * cos).astype(np.float32)
    srT = np.ascontiguousarray(sr.T)  # [D, P]
    siT = np.ascontiguousarray(si.T)
    ssT = srT + siT

    in_maps = []
    for c in range(NCORES):
        rows = slice(c * BC, (c + 1) * BC)
        xs = x_real[rows]
        xis = x_imag[rows]
        in_maps.append(
            {
                "xrT": np.ascontiguousarray(xs.T),
                "xiT": np.ascontiguousarray(xis.T),
                "wT": np.ascontiguousarray(xs.T - xis.T),
                "srT": srT,
                "siT": siT,
                "ssT": ssT,
            }
        )
    return in_maps


def run(inputs, trace=False, **trace_kwargs):
    """Build + run on 8 cores. Returns (full_output, BassKernelResults)."""
    in_maps = _prepare_in_maps(**inputs)
    nc = build_program()
    res = run_bass_kernel_spmd(
        nc, in_maps, list(range(NCORES)), trace=trace, **trace_kwargs
    )
    full = np.concatenate(
        [np.ascontiguousarray(r["out"].T) for r in res.results], axis=0
    )
    return full, res


def kernel(x_real, x_imag, patterns_real, patterns_imag, phases):
    full, _ = run(
        {
            "x_real": x_real,
            "x_imag": x_imag,
            "patterns_real": patterns_real,
            "patterns_imag": patterns_imag,
            "phases": phases,
        }
    )
    return full
